# revision 7
# baseline (speedup 1.0000x reference)
"""Trainium2 Bass kernel for nn_NDConv: 3x3 valid conv2d, Cin=Cout=64.

The reference's FFT conv is mathematically a standard valid cross-correlation:
  out[b, co, p, q] = sum_{ci,ky,kx} weight[co, ci, ky, kx] * x[b, ci, p+ky, q+kx] + bias[co]
with x [8, 64, 256, 256], weight [64, 64, 3, 3] -> out [8, 64, 254, 254].

Sharding: data-parallel over batch, one image per NeuronCore (8 cores).

Per-core kernel: 3x3 conv as 9 shifted matmuls on the PE array, with K-dim
packing: SBUF partitions 0-63 hold x[ci, r, :], partitions 64-127 hold the
row-shifted replica x[ci, r+1, :], so the ky=0 and ky=1 taps fuse into one
K=128 matmul (3 of them, one per kx) and ky=2 runs as K=64 (3 more).
PSUM tile [128, 2, 256]: partitions 0-63 and 64-127 carry two independent
2-output-row groups (bank-exact 512 fp32). float32r gives fp32-precision-ish
matmul at 1 cycle/row (4x faster than plain fp32). Bias is fused into the
PSUM->SBUF eviction on the vector engine; output stores go through the
activation engine's HWDGE queue so they never stall the x-strip loads on SP.
"""

import numpy as np
from contextlib import ExitStack

from concourse import bacc, mybir
import concourse.tile as tile
from concourse.bass_utils import run_bass_kernel_spmd

F32 = mybir.dt.float32
F32R = mybir.dt.float32r

CI = 64
CO = 64
H = W = 256
KH = KW = 3
OH = OW = 254
NCORES = 8
STRIP = 64  # output rows per strip (last strip is 62)

_cache = {}


def build_conv_bass():
    nc = bacc.Bacc("TRN2", target_bir_lowering=False, debug=False)

    x_d = nc.dram_tensor("x", [CI, H, W], F32, kind="ExternalInput").ap()
    wt_d = nc.dram_tensor("wt", [KH, KW, CI, CO], F32, kind="ExternalInput").ap()
    b_d = nc.dram_tensor("bias", [CO, 1], F32, kind="ExternalInput").ap()
    out_d = nc.dram_tensor("out", [CO, OH, OW], F32, kind="ExternalOutput").ap()

    with tile.TileContext(nc) as tc, ExitStack() as ctx:
        wpool = ctx.enter_context(tc.tile_pool(name="wpool", bufs=1))
        xpool = ctx.enter_context(tc.tile_pool(name="xpool", bufs=2))
        stpool = ctx.enter_context(tc.tile_pool(name="stpool", bufs=4))
        pspool = ctx.enter_context(tc.tile_pool(name="pspool", bufs=6, space="PSUM"))

        # Weights, transposed on host to wt[ky, kx, ci, co].
        # wpair[dx]: rows 0-63 = tap (ky=0, dx), rows 64-127 = tap (ky=1, dx).
        wpair = []
        wtop = []
        for dx in range(KW):
            wp = wpool.tile([2 * CI, CO], F32R, name=f"wp{dx}", tag=f"wp{dx}")
            nc.sync.dma_start(wp[0:CI, :], wt_d[0, dx].bitcast(F32R))
            nc.sync.dma_start(wp[CI : 2 * CI, :], wt_d[1, dx].bitcast(F32R))
            wpair.append(wp)
            wtt = wpool.tile([CI, CO], F32R, name=f"wt{dx}", tag=f"wt{dx}")
            nc.sync.dma_start(wtt[:, :], wt_d[2, dx].bitcast(F32R))
            wtop.append(wtt)
        btile = wpool.tile([2 * CO, 1], F32, name="btile", tag="btile")
        nc.sync.dma_start(btile[0:CO, :], b_d[:, :])
        nc.sync.dma_start(btile[CO : 2 * CO, :], b_d[:, :])

        strips = []
        p0 = 0
        while p0 < OH:
            strips.append((p0, min(STRIP, OH - p0)))
            p0 += STRIP

        for p0, S in strips:
            # x strip: lower 64 partitions = x rows [p0, p0+S+2);
            # upper 64 partitions = x rows [p0+1, p0+S+2) (the +1-row replica).
            xt = xpool.tile([128, S + 2, W], F32R, name="xt", tag="xt")
            nc.sync.dma_start(
                xt[0:CI, 0 : S + 2, :], x_d[:, p0 : p0 + S + 2, :].bitcast(F32R)
            )
            nc.sync.dma_start(
                xt[CI:128, 0 : S + 1, :], x_d[:, p0 + 1 : p0 + S + 2, :].bitcast(F32R)
            )

            for base in range(0, S, 2):  # 2 output rows per PSUM bank
                # One PSUM bank: 512 fp32/partition; rows packed as 2*254=508.
                ps = pspool.tile([128, 512], F32, name="ps", tag="ps")
                osl = ps[0:CO, 0 : 2 * OW]
                k = 0
                for dx in range(KW):  # taps (0,dx)+(1,dx), K=128
                    rhs = xt[:, base : base + 2, dx : dx + OW]
                    nc.tensor.matmul(
                        osl, wpair[dx][:], rhs, start=(k == 0), stop=(k == 5)
                    )
                    k += 1
                for dx in range(KW):  # tap (2,dx), K=64
                    rhs = xt[0:CI, base + 2 : base + 4, dx : dx + OW]
                    nc.tensor.matmul(
                        osl, wtop[dx][:], rhs, start=(k == 0), stop=(k == 5)
                    )
                    k += 1

                stage = stpool.tile([128, 2 * OW], F32, name="stage", tag="st")
                nc.vector.tensor_scalar_add(
                    stage[0:CO, :], ps[0:CO, 0 : 2 * OW], btile[0:CO, :]
                )
                r0 = p0 + base
                nc.scalar.dma_start(
                    out_d[:, r0 : r0 + 2, :].rearrange("c r q -> c (r q)"),
                    stage[0:CO, :],
                )

    nc.compile()
    return nc


def kernel(x, weight, bias):
    x = np.asarray(x, dtype=np.float32)
    weight = np.asarray(weight, dtype=np.float32)
    bias = np.asarray(bias, dtype=np.float32)

    if "nc" not in _cache:
        _cache["nc"] = build_conv_bass()
    nc = _cache["nc"]

    wt = np.ascontiguousarray(weight.transpose(2, 3, 1, 0))  # [ky, kx, ci, co]
    b2 = np.ascontiguousarray(bias.reshape(CO, 1))
    in_maps = [
        {"x": np.ascontiguousarray(x[i]), "wt": wt, "bias": b2} for i in range(NCORES)
    ]
    res = run_bass_kernel_spmd(nc, in_maps, list(range(NCORES)))
    out = np.stack([res.results[i]["out"] for i in range(NCORES)], axis=0)
    return out.astype(np.float32)


# revision 8
# speedup vs baseline: 1.0859x; 1.0859x over previous
"""Trainium2 Bass kernel for nn_NDConv: 3x3 valid conv2d, Cin=Cout=64.

The reference's FFT conv is mathematically a standard valid cross-correlation:
  out[b, co, p, q] = sum_{ci,ky,kx} weight[co, ci, ky, kx] * x[b, ci, p+ky, q+kx] + bias[co]
with x [8, 64, 256, 256], weight [64, 64, 3, 3] -> out [8, 64, 254, 254].

Sharding: data-parallel over batch, one image per NeuronCore (8 cores).

Per-core kernel: 3x3 conv as 9 shifted matmuls on the PE array, with K-dim
packing: SBUF partitions 0-63 hold x[ci, r, :], partitions 64-127 hold the
row-shifted replica x[ci, r+1, :], so the ky=0 and ky=1 taps fuse into one
K=128 matmul (3 of them, one per kx) and ky=2 runs as K=64 (3 more).
PSUM tile [128, 2, 256]: partitions 0-63 and 64-127 carry two independent
2-output-row groups (bank-exact 512 fp32). float32r gives fp32-precision-ish
matmul at 1 cycle/row (4x faster than plain fp32). Bias is fused into the
PSUM->SBUF eviction on the vector engine; output stores go through the
activation engine's HWDGE queue so they never stall the x-strip loads on SP.
"""

import numpy as np
from contextlib import ExitStack

from concourse import bacc, mybir
import concourse.tile as tile
from concourse.bass_utils import run_bass_kernel_spmd

F32 = mybir.dt.float32
F32R = mybir.dt.float32r
F16 = mybir.dt.float16

CI = 64
CO = 64
H = W = 256
KH = KW = 3
OH = OW = 254
NCORES = 8
STRIP = 64  # output rows per strip (last strip is 62)

_cache = {}


def build_conv_bass():
    nc = bacc.Bacc("TRN2", target_bir_lowering=False, debug=False)

    x_d = nc.dram_tensor("x", [CI, H, W], F16, kind="ExternalInput").ap()
    wt_d = nc.dram_tensor("wt", [KH, KW, CI, CO], F16, kind="ExternalInput").ap()
    b_d = nc.dram_tensor("bias", [CO, 1], F32, kind="ExternalInput").ap()
    out_d = nc.dram_tensor("out", [CO, OH, OW], F32, kind="ExternalOutput").ap()

    with tile.TileContext(nc) as tc, ExitStack() as ctx:
        wpool = ctx.enter_context(tc.tile_pool(name="wpool", bufs=1))
        xpool = ctx.enter_context(tc.tile_pool(name="xpool", bufs=2))
        stpool = ctx.enter_context(tc.tile_pool(name="stpool", bufs=4))
        pspool = ctx.enter_context(tc.tile_pool(name="pspool", bufs=6, space="PSUM"))

        # Weights, transposed on host to wt[ky, kx, ci, co].
        # wpair[dx]: rows 0-63 = tap (ky=0, dx), rows 64-127 = tap (ky=1, dx).
        wpair = []
        wtop = []
        for dx in range(KW):
            wp = wpool.tile([2 * CI, CO], F16, name=f"wp{dx}", tag=f"wp{dx}")
            nc.sync.dma_start(wp[0:CI, :], wt_d[0, dx])
            nc.sync.dma_start(wp[CI : 2 * CI, :], wt_d[1, dx])
            wpair.append(wp)
            wtt = wpool.tile([CI, CO], F16, name=f"wt{dx}", tag=f"wt{dx}")
            nc.sync.dma_start(wtt[:, :], wt_d[2, dx])
            wtop.append(wtt)
        btile = wpool.tile([2 * CO, 1], F32, name="btile", tag="btile")
        nc.sync.dma_start(btile[0:CO, :], b_d[:, :])
        nc.sync.dma_start(btile[CO : 2 * CO, :], b_d[:, :])

        strips = []
        p0 = 0
        while p0 < OH:
            strips.append((p0, min(STRIP, OH - p0)))
            p0 += STRIP

        for p0, S in strips:
            # x strip: lower 64 partitions = x rows [p0, p0+S+2);
            # upper 64 partitions = x rows [p0+1, p0+S+2) (the +1-row replica).
            xt = xpool.tile([128, S + 2, W], F16, name="xt", tag="xt")
            nc.sync.dma_start(
                xt[0:CI, 0 : S + 2, :], x_d[:, p0 : p0 + S + 2, :]
            )
            nc.sync.dma_start(
                xt[CI:128, 0 : S + 1, :], x_d[:, p0 + 1 : p0 + S + 2, :]
            )

            for base in range(0, S, 2):  # 2 output rows per PSUM bank
                # One PSUM bank: 512 fp32/partition; rows packed as 2*254=508.
                ps = pspool.tile([128, 512], F32, name="ps", tag="ps")
                osl = ps[0:CO, 0 : 2 * OW]
                k = 0
                for dx in range(KW):  # taps (0,dx)+(1,dx), K=128
                    rhs = xt[:, base : base + 2, dx : dx + OW]
                    nc.tensor.matmul(
                        osl, wpair[dx][:], rhs, start=(k == 0), stop=(k == 5)
                    )
                    k += 1
                for dx in range(KW):  # tap (2,dx), K=64
                    rhs = xt[0:CI, base + 2 : base + 4, dx : dx + OW]
                    nc.tensor.matmul(
                        osl, wtop[dx][:], rhs, start=(k == 0), stop=(k == 5)
                    )
                    k += 1

                stage = stpool.tile([128, 2 * OW], F32, name="stage", tag="st")
                nc.vector.tensor_scalar_add(
                    stage[0:CO, :], ps[0:CO, 0 : 2 * OW], btile[0:CO, :]
                )
                r0 = p0 + base
                nc.scalar.dma_start(
                    out_d[:, r0 : r0 + 2, :].rearrange("c r q -> c (r q)"),
                    stage[0:CO, :],
                )

    nc.compile()
    return nc


def kernel(x, weight, bias):
    x = np.asarray(x, dtype=np.float32)
    weight = np.asarray(weight, dtype=np.float32)
    bias = np.asarray(bias, dtype=np.float32)

    if "nc" not in _cache:
        _cache["nc"] = build_conv_bass()
    nc = _cache["nc"]

    wt = np.ascontiguousarray(weight.transpose(2, 3, 1, 0).astype(np.float16))
    b2 = np.ascontiguousarray(bias.reshape(CO, 1))
    in_maps = [
        {"x": np.ascontiguousarray(x[i].astype(np.float16)), "wt": wt, "bias": b2}
        for i in range(NCORES)
    ]
    res = run_bass_kernel_spmd(nc, in_maps, list(range(NCORES)))
    out = np.stack([res.results[i]["out"] for i in range(NCORES)], axis=0)
    return out.astype(np.float32)


# revision 10
# speedup vs baseline: 1.5117x; 1.3920x over previous
"""Trainium2 Bass kernel for nn_NDConv: 3x3 valid conv2d, Cin=Cout=64.

The reference's FFT conv is mathematically a standard valid cross-correlation:
  out[b, co, p, q] = sum_{ci,ky,kx} weight[co, ci, ky, kx] * x[b, ci, p+ky, q+kx] + bias[co]
with x [8, 64, 256, 256], weight [64, 64, 3, 3] -> out [8, 64, 254, 254].

Sharding: data-parallel over batch, one image per NeuronCore (8 cores).

Per-core kernel: 3x3 conv as 9 shifted matmuls on the PE array, with K-dim
packing: SBUF partitions 0-63 hold x[ci, r, :], partitions 64-127 hold the
row-shifted replica x[ci, r+1, :], so the ky=0 and ky=1 taps fuse into one
K=128 matmul (3 of them, one per kx) and ky=2 runs as K=64 (3 more).
PSUM tile [128, 2, 256]: partitions 0-63 and 64-127 carry two independent
2-output-row groups (bank-exact 512 fp32). float32r gives fp32-precision-ish
matmul at 1 cycle/row (4x faster than plain fp32). Bias is fused into the
PSUM->SBUF eviction on the vector engine; output stores go through the
activation engine's HWDGE queue so they never stall the x-strip loads on SP.
"""

import numpy as np
from contextlib import ExitStack

from concourse import bacc, mybir
import concourse.tile as tile
from concourse.bass_utils import run_bass_kernel_spmd

F32 = mybir.dt.float32
F32R = mybir.dt.float32r
F16 = mybir.dt.float16

CI = 64
CO = 64
H = W = 256
KH = KW = 3
OH = OW = 254
NCORES = 8
STRIP = 64  # output rows per strip (last strip is 62)

_cache = {}


def build_conv_bass():
    nc = bacc.Bacc("TRN2", target_bir_lowering=False, debug=False)

    x_d = nc.dram_tensor("x", [CI, H, W], F16, kind="ExternalInput").ap()
    wt_d = nc.dram_tensor("wt", [KH, KW, CI, CO], F16, kind="ExternalInput").ap()
    b_d = nc.dram_tensor("bias", [CO, 1], F32, kind="ExternalInput").ap()
    out_d = nc.dram_tensor("out", [CO, OH, OW], F32, kind="ExternalOutput").ap()

    with tile.TileContext(nc) as tc, ExitStack() as ctx:
        wpool = ctx.enter_context(tc.tile_pool(name="wpool", bufs=1))
        xpool = ctx.enter_context(tc.tile_pool(name="xpool", bufs=2))
        stpool = ctx.enter_context(tc.tile_pool(name="stpool", bufs=4))
        pspool = ctx.enter_context(tc.tile_pool(name="pspool", bufs=6, space="PSUM"))

        # Weights, transposed on host to wt[ky, kx, ci, co].
        # wpair[dx]: rows 0-63 = tap (ky=0, dx), rows 64-127 = tap (ky=1, dx).
        wpair = []
        wtop = []
        for dx in range(KW):
            wp = wpool.tile([2 * CI, CO], F16, name=f"wp{dx}", tag=f"wp{dx}")
            nc.sync.dma_start(wp[0:CI, :], wt_d[0, dx])
            nc.sync.dma_start(wp[CI : 2 * CI, :], wt_d[1, dx])
            wpair.append(wp)
            # ky=2 taps as K=128 with zero weights in rows 64-127: keeping every
            # matmul at tile_size (128, 64) keeps the PE at full clock (mixing
            # 128/64-row tiles locks the PE at half frequency).
            wtt = wpool.tile([2 * CI, CO], F16, name=f"wt{dx}", tag=f"wt{dx}")
            nc.sync.dma_start(wtt[0:CI, :], wt_d[2, dx])
            nc.gpsimd.memset(wtt[CI : 2 * CI, :], 0.0)
            wtop.append(wtt)
        btile = wpool.tile([2 * CO, 1], F32, name="btile", tag="btile")
        nc.sync.dma_start(btile[0:CO, :], b_d[:, :])
        nc.sync.dma_start(btile[CO : 2 * CO, :], b_d[:, :])

        strips = []
        p0 = 0
        while p0 < OH:
            strips.append((p0, min(STRIP, OH - p0)))
            p0 += STRIP

        for p0, S in strips:
            # x strip: lower 64 partitions = x rows [p0, p0+S+2);
            # upper 64 partitions = x rows [p0+1, p0+S+2) (the +1-row replica).
            xt = xpool.tile([128, S + 2, W], F16, name="xt", tag="xt")
            nc.sync.dma_start(
                xt[0:CI, 0 : S + 2, :], x_d[:, p0 : p0 + S + 2, :]
            )
            # Upper rows feed the zero-weight half of the ky=2 matmuls too, so
            # fill all S+2 slots when the source row exists; memset the last
            # slot on the final strip (row 256 doesn't exist).
            ru = min(S + 2, H - p0 - 1)
            nc.sync.dma_start(
                xt[CI:128, 0:ru, :], x_d[:, p0 + 1 : p0 + 1 + ru, :]
            )
            if ru < S + 2:
                nc.gpsimd.memset(xt[CI:128, ru : S + 2, :], 0.0)

            for base in range(0, S, 2):  # 2 output rows per PSUM bank
                # One PSUM bank: 512 fp32/partition; rows packed as 2*254=508.
                ps = pspool.tile([128, 512], F32, name="ps", tag="ps")
                osl = ps[0:CO, 0 : 2 * OW]
                k = 0
                for dx in range(KW):  # taps (0,dx)+(1,dx), K=128
                    rhs = xt[:, base : base + 2, dx : dx + OW]
                    nc.tensor.matmul(
                        osl, wpair[dx][:], rhs, start=(k == 0), stop=(k == 5)
                    )
                    k += 1
                for dx in range(KW):  # tap (2,dx), K=128 zero-padded
                    rhs = xt[:, base + 2 : base + 4, dx : dx + OW]
                    nc.tensor.matmul(
                        osl, wtop[dx][:], rhs, start=(k == 0), stop=(k == 5)
                    )
                    k += 1

                stage = stpool.tile([128, 2 * OW], F32, name="stage", tag="st")
                nc.vector.tensor_scalar_add(
                    stage[0:CO, :], ps[0:CO, 0 : 2 * OW], btile[0:CO, :]
                )
                r0 = p0 + base
                nc.scalar.dma_start(
                    out_d[:, r0 : r0 + 2, :].rearrange("c r q -> c (r q)"),
                    stage[0:CO, :],
                )

    nc.compile()
    return nc


def kernel(x, weight, bias):
    x = np.asarray(x, dtype=np.float32)
    weight = np.asarray(weight, dtype=np.float32)
    bias = np.asarray(bias, dtype=np.float32)

    if "nc" not in _cache:
        _cache["nc"] = build_conv_bass()
    nc = _cache["nc"]

    wt = np.ascontiguousarray(weight.transpose(2, 3, 1, 0).astype(np.float16))
    b2 = np.ascontiguousarray(bias.reshape(CO, 1))
    in_maps = [
        {"x": np.ascontiguousarray(x[i].astype(np.float16)), "wt": wt, "bias": b2}
        for i in range(NCORES)
    ]
    res = run_bass_kernel_spmd(nc, in_maps, list(range(NCORES)))
    out = np.stack([res.results[i]["out"] for i in range(NCORES)], axis=0)
    return out.astype(np.float32)


# revision 13
# speedup vs baseline: 2.1116x; 1.3968x over previous
"""Trainium2 Bass kernel for nn_NDConv: 3x3 valid conv2d, Cin=Cout=64.

The reference's FFT conv is mathematically a standard valid cross-correlation:
  out[b, co, p, q] = sum_{ci,ky,kx} weight[co, ci, ky, kx] * x[b, ci, p+ky, q+kx] + bias[co]
with x [8, 64, 256, 256], weight [64, 64, 3, 3] -> out [8, 64, 254, 254].

Sharding: data-parallel over batch, one image per NeuronCore (8 cores).

Per-core kernel: 3x3 conv as 9 shifted matmuls on the PE array, with K-dim
packing: SBUF partitions 0-63 hold x[ci, r, :], partitions 64-127 hold the
row-shifted replica x[ci, r+1, :], so the ky=0 and ky=1 taps fuse into one
K=128 matmul (3 of them, one per kx) and ky=2 runs as K=64 (3 more).
PSUM tile [128, 2, 256]: partitions 0-63 and 64-127 carry two independent
2-output-row groups (bank-exact 512 fp32). float32r gives fp32-precision-ish
matmul at 1 cycle/row (4x faster than plain fp32). Bias is fused into the
PSUM->SBUF eviction on the vector engine; output stores go through the
activation engine's HWDGE queue so they never stall the x-strip loads on SP.
"""

import numpy as np
from contextlib import ExitStack

from concourse import bacc, mybir
import concourse.tile as tile
from concourse.bass_utils import run_bass_kernel_spmd

F32 = mybir.dt.float32
F32R = mybir.dt.float32r
F16 = mybir.dt.float16

CI = 64
CO = 64
H = W = 256
KH = KW = 3
OH = OW = 254
NCORES = 8
STRIP = 16  # output rows per x chunk (last chunk is 14)

_cache = {}


def build_conv_bass():
    nc = bacc.Bacc("TRN2", target_bir_lowering=False, debug=False)

    x_d = nc.dram_tensor("x", [CI, H, W], F16, kind="ExternalInput").ap()
    wt_d = nc.dram_tensor("wt", [KH, KW, CI, CO], F16, kind="ExternalInput").ap()
    b_d = nc.dram_tensor("bias", [CO, 1], F32, kind="ExternalInput").ap()
    out_d = nc.dram_tensor("out", [CO, OH, OW], F32, kind="ExternalOutput").ap()

    with tile.TileContext(nc) as tc, ExitStack() as ctx:
        wpool = ctx.enter_context(tc.tile_pool(name="wpool", bufs=1))
        xpool = ctx.enter_context(tc.tile_pool(name="xpool", bufs=3))
        stpool = ctx.enter_context(tc.tile_pool(name="stpool", bufs=4))
        pspool = ctx.enter_context(tc.tile_pool(name="pspool", bufs=8, space="PSUM"))

        # Weights, transposed on host to wt[ky, kx, ci, co].
        # wpair[dx]: rows 0-63 = tap (ky=0, dx), rows 64-127 = tap (ky=1, dx).
        wpair = []
        wtop = []
        for dx in range(KW):
            wp = wpool.tile([2 * CI, CO], F16, name=f"wp{dx}", tag=f"wp{dx}")
            nc.sync.dma_start(wp[0:CI, :], wt_d[0, dx])
            nc.sync.dma_start(wp[CI : 2 * CI, :], wt_d[1, dx])
            wpair.append(wp)
            # ky=2 taps as K=128 with zero weights in rows 64-127: keeping every
            # matmul at tile_size (128, 64) keeps the PE at full clock (mixing
            # 128/64-row tiles locks the PE at half frequency).
            wtt = wpool.tile([2 * CI, CO], F16, name=f"wt{dx}", tag=f"wt{dx}")
            nc.sync.dma_start(wtt[0:CI, :], wt_d[2, dx])
            nc.gpsimd.memset(wtt[CI : 2 * CI, :], 0.0)
            wtop.append(wtt)
        btile = wpool.tile([2 * CO, 1], F32, name="btile", tag="btile")
        nc.sync.dma_start(btile[0:CO, :], b_d[:, :])
        nc.sync.dma_start(btile[CO : 2 * CO, :], b_d[:, :])

        chunks = []
        p0 = 0
        while p0 < OH:
            chunks.append((p0, min(STRIP, OH - p0)))
            p0 += STRIP

        for p0, S in chunks:
            # x chunk: lower 64 partitions = x rows [p0, p0+S+2);
            # upper 64 partitions = x rows [p0+1, p0+S+3) (the +1-row replica,
            # which also feeds the zero-weight half of the ky=2 matmuls).
            xt = xpool.tile([128, STRIP + 2, W], F16, name="xt", tag="xt")
            nc.sync.dma_start(
                xt[0:CI, 0 : S + 2, :], x_d[:, p0 : p0 + S + 2, :]
            )
            ru = min(S + 2, H - p0 - 1)
            nc.sync.dma_start(
                xt[CI:128, 0:ru, :], x_d[:, p0 + 1 : p0 + 1 + ru, :]
            )
            if ru < S + 2:
                nc.gpsimd.memset(xt[CI:128, ru : S + 2, :], 0.0)

            for base in range(0, S, 2):  # 2 output rows per PSUM bank
                # One PSUM bank: 512 fp32/partition; rows packed as 2*254=508.
                ps = pspool.tile([128, 512], F32, name="ps", tag="ps")
                osl = ps[0:CO, 0 : 2 * OW]
                k = 0
                for dx in range(KW):  # taps (0,dx)+(1,dx), K=128
                    rhs = xt[:, base : base + 2, dx : dx + OW]
                    nc.tensor.matmul(
                        osl, wpair[dx][:], rhs, start=(k == 0), stop=(k == 5)
                    )
                    k += 1
                for dx in range(KW):  # tap (2,dx), K=128 zero-padded
                    rhs = xt[:, base + 2 : base + 4, dx : dx + OW]
                    nc.tensor.matmul(
                        osl, wtop[dx][:], rhs, start=(k == 0), stop=(k == 5)
                    )
                    k += 1

                stage = stpool.tile([128, 2 * OW], F32, name="stage", tag="st")
                nc.vector.tensor_scalar_add(
                    stage[0:CO, :], ps[0:CO, 0 : 2 * OW], btile[0:CO, :]
                )
                r0 = p0 + base
                nc.scalar.dma_start(
                    out_d[:, r0 : r0 + 2, :].rearrange("c r q -> c (r q)"),
                    stage[0:CO, :],
                )

    nc.compile()
    return nc


def kernel(x, weight, bias):
    x = np.asarray(x, dtype=np.float32)
    weight = np.asarray(weight, dtype=np.float32)
    bias = np.asarray(bias, dtype=np.float32)

    if "nc" not in _cache:
        _cache["nc"] = build_conv_bass()
    nc = _cache["nc"]

    wt = np.ascontiguousarray(weight.transpose(2, 3, 1, 0).astype(np.float16))
    b2 = np.ascontiguousarray(bias.reshape(CO, 1))
    in_maps = [
        {"x": np.ascontiguousarray(x[i].astype(np.float16)), "wt": wt, "bias": b2}
        for i in range(NCORES)
    ]
    res = run_bass_kernel_spmd(nc, in_maps, list(range(NCORES)))
    out = np.stack([res.results[i]["out"] for i in range(NCORES)], axis=0)
    return out.astype(np.float32)
